# revision 26
# baseline (speedup 1.0000x reference)
"""Trainium2 Bass kernel for BlockAxialDown (maxpool + axial attention + 1x1 conv + batchnorm).

Contract: kernel(**inputs) takes FULL unsharded inputs, returns FULL output.
Sharding: data-parallel over batch B=8 across 8 NeuronCores (1 image/core);
BatchNorm batch stats combined with a tiny (128,4) AllReduce; weights replicated.

Design (vs the DMA-transpose baseline):
- Attention computed transpose-free: dots built in (j,i) orientation as
  dT = k_slice.T @ q_slice (64-contraction, head pair packed into opposite
  PE row-quadrants so the two matmuls run concurrently), and the AV product
  oT = v.T @ e consumes e directly — no DMA transposes (baseline spent
  610us serialized on them).
- Softmax denominators broadcast across partitions by a ones-weights matmul
  (every output row of ones64.T @ e is the column sum).
- Softmax division deferred: fronts store unnormalized oT (bf16) and sums
  (f16) in SBUF; after all 32 groups, ONE big Ln instruction then per-group
  exp(-ln) -> multiply -> out-projection. One ACT table switch per direction
  instead of 128 (each switch costs ~1.5us), and the PE front stays dense.
- exp batched to one ACT instruction per group (FD=1024).
- H-direction attention reads a transposed copy xpT built by DVE strided
  copies (hits the fast copy mode; ACT is 5x slower on strided input).
- Conv pass 2 eliminated: pass-1 relu output stored bf16 in SBUF (reusing
  the maxpool/xpT scratch regions), final pass is affine-only + DMA out,
  split across DVE/ACT.
"""

import sys

import numpy as np

for _p in ("/opt/trn_rl_repo", "/root/.axon_site/_ro/trn_rl_repo"):
    if _p not in sys.path:
        sys.path.append(_p)

B, C, H, W = 8, 128, 256, 256
H2, W2 = 128, 128
E = 2 * C
NPOS = H2 * W2
NCORES = 8
BN_EPS = 1e-5
DH = C // 2
SCALE = DH ** -0.5

_CACHE = {}


def _build_program():
    import concourse.tile as tile
    from concourse import bacc, mybir
    from concourse.alu_op_type import AluOpType
    from contextlib import ExitStack

    F32 = mybir.dt.float32
    BF16 = mybir.dt.bfloat16
    F16 = mybir.dt.float16
    AF = mybir.ActivationFunctionType
    P = 128

    nc = bacc.Bacc("TRN2", target_bir_lowering=False, debug=False, num_devices=NCORES)

    # ---- DRAM I/O ----
    x_d = nc.dram_tensor("x", [C, H, W], BF16, kind="ExternalInput").ap()
    wq_w_d = nc.dram_tensor("wq_w", [C, C], BF16, kind="ExternalInput").ap()
    wk_w_d = nc.dram_tensor("wk_w", [C, C], BF16, kind="ExternalInput").ap()
    wq_h_d = nc.dram_tensor("wq_h", [C, C], BF16, kind="ExternalInput").ap()
    wk_h_d = nc.dram_tensor("wk_h", [C, C], BF16, kind="ExternalInput").ap()
    wv_w_d = nc.dram_tensor("wv_w", [C, C], BF16, kind="ExternalInput").ap()
    wo_w_d = nc.dram_tensor("wo_w", [C, C], BF16, kind="ExternalInput").ap()
    wv_h_d = nc.dram_tensor("wv_h", [C, C], BF16, kind="ExternalInput").ap()
    wo_h_d = nc.dram_tensor("wo_h", [C, C], BF16, kind="ExternalInput").ap()
    bsum_d = nc.dram_tensor("bsum", [C, 1], F32, kind="ExternalInput").ap()
    convA_d = nc.dram_tensor("convA", [C, E], BF16, kind="ExternalInput").ap()
    convX_d = nc.dram_tensor("convX", [C, E], BF16, kind="ExternalInput").ap()
    gamma2_d = nc.dram_tensor("gamma2", [C, 2], F32, kind="ExternalInput").ap()
    beta2_d = nc.dram_tensor("beta2", [C, 2], F32, kind="ExternalInput").ap()
    out_d = nc.dram_tensor("out", [E, H2, W2], F32, kind="ExternalOutput").ap()
    stats_in_d = nc.dram_tensor("stats_in", [P, 4], F32).ap()
    stats_out_d = nc.dram_tensor("stats_out", [P, 4], F32, addr_space="Shared").ap()

    with tile.TileContext(nc) as tc, ExitStack() as ctx:
        const = ctx.enter_context(tc.tile_pool(name="const", bufs=1))
        cube = ctx.enter_context(tc.tile_pool(name="cube", bufs=1))
        stage = ctx.enter_context(tc.tile_pool(name="stage", bufs=3))
        work = ctx.enter_context(tc.tile_pool(name="work", bufs=2))
        stats = ctx.enter_context(tc.tile_pool(name="stats", bufs=1))
        psum = ctx.enter_context(tc.tile_pool(name="psum", bufs=1, space="PSUM"))

        # ---- constants ----
        def cload(name, ap_d, shape, dt):
            t = const.tile(shape, dt, name=name)
            nc.sync.dma_start(out=t[:], in_=ap_d)
            return t

        m_w = [cload("wq_w_t", wq_w_d, [C, C], BF16),
               cload("wk_w_t", wk_w_d, [C, C], BF16)]
        m_h = [cload("wq_h_t", wq_h_d, [C, C], BF16),
               cload("wk_h_t", wk_h_d, [C, C], BF16)]
        wv_w = cload("wv_w_t", wv_w_d, [C, C], BF16)
        wo_w = cload("wo_w_t", wo_w_d, [C, C], BF16)
        wv_h = cload("wv_h_t", wv_h_d, [C, C], BF16)
        wo_h = cload("wo_h_t", wo_h_d, [C, C], BF16)
        bsum = cload("bsum_t", bsum_d, [C, 1], F32)
        convA = cload("convA_t", convA_d, [C, E], BF16)
        convX = cload("convX_t", convX_d, [C, E], BF16)
        gamma2 = cload("gamma2_t", gamma2_d, [C, 2], F32)
        beta2 = cload("beta2_t", beta2_d, [C, 2], F32)
        ones64 = const.tile([P, 64], BF16, name="ones64")
        nc.vector.memset(ones64[:], 1.0)

        xp = cube.tile([P, H2, W2], BF16)   # pooled input, channels on partitions
        acc = cube.tile([P, H2, W2], BF16)  # attention output accumulator
        S = cube.tile([P, 2 * NPOS], BF16)  # scratch: xw -> xpT -> ystore
        xp_f = xp[:].rearrange("c h w -> c (h w)")
        acc_f = acc[:].rearrange("c h w -> c (h w)")
        xw_v = S[:].rearrange("c (h w) -> c h w", w=W)        # (c, 128, 256)
        xpT_v = S[:, 0:NPOS].rearrange("c (w h) -> c w h", h=H2)  # (c, 128, 128)
        y_v = S[:].rearrange("c (e n) -> c e n", e=2)         # ystore halves

        # ---- phase 1: load + 2x2 maxpool (both maxes on DVE) ----
        xv = x_d.rearrange("c (n h) w -> c n h w", h=8)
        for i in range(H // 8):
            xin = stage.tile([P, 8, W], BF16, tag="xin")
            nc.sync.dma_start(out=xin[:], in_=xv[:, i])
            xw_c = xw_v[:, 4 * i:4 * i + 4, :]
            xin_v = xin[:].rearrange("c (r two) w -> c r two w", two=2)
            nc.vector.tensor_max(xw_c, xin_v[:, :, 0, :], xin_v[:, :, 1, :])
            xw4 = xw_c.rearrange("c r (w two) -> c r w two", two=2)
            nc.vector.tensor_max(xp[:, 4 * i:4 * i + 4, :],
                                 xw4[:, :, :, 0], xw4[:, :, :, 1])

        # ---- axial attention, normalization deferred ----
        # Front half per 4-slice group: projections, transposed dots, exp,
        # ones-matmul column sums, unnormalized AV. Stores oT (bf16) and sums
        # (f16) to SBUF. Softmax division is deferred to a batched pass so the
        # two ACT table sets (exp / ln) each load once per direction instead
        # of thrashing every group, and the PE front stays dense (HAM warm).
        oTs_v = S[:, NPOS:2 * NPOS].rearrange("c (g i) -> c g i", i=512)
        sums_st = cube.tile([P, 32, 512], F16, name="sums_st")

        def front_group(g, xg, m01, wv):
            xg_f = xg.rearrange("c s i -> c (s i)")
            # q/k projections: qk cols [0:512]=q, [512:1024]=k. Per-half PSUM
            # tiles (bufs=2) + per-half casts so the next group's projection
            # matmuls are not serialized behind this group's full cast.
            mk = work.tile([P, 1024], BF16, tag="mk", bufs=2)
            for h in range(2):
                mkps = psum.tile([P, 512], F32, tag="mkps", bufs=2, name="mkps")
                nc.tensor.matmul(mkps[:], lhsT=m01[h][:],
                                 rhs=xg_f, start=True, stop=True)
                nc.scalar.copy(mk[:, 512 * h:512 * h + 512], mkps[:])
            # v projection per slice: (j, d) layout
            vps = psum.tile([P, 512], F32, tag="vps", bufs=1, name="vps")
            for s in range(4):
                nc.tensor.matmul(vps[:, 128 * s:128 * s + 128], lhsT=xg[:, s, :],
                                 rhs=wv[:], start=True, stop=True)
            vs = work.tile([P, 512], BF16, tag="vs", bufs=2)
            nc.vector.tensor_copy(vs[:], vps[:])
            # dots transposed: dT[j, i] = k_j . q_i ; blocks (h, s).
            # 64-contraction per head, packed into opposite PE row-quadrants
            # (base_partition 0/64) so the head pair runs concurrently and
            # LDWEIGHTS overlaps in-flight matmuls.
            e = work.tile([P, 1024], BF16, tag="e", bufs=2)
            for h in range(2):
                dT = psum.tile([P, 512], F32, tag="dT", bufs=2, name="dT")
                for s in range(4):
                    hp = slice(64 * h, 64 * h + 64)
                    qs = mk[hp, 128 * s:128 * s + 128]
                    ks = mk[hp, 512 + 128 * s:512 + 128 * s + 128]
                    nc.tensor.matmul(dT[:, 128 * s:128 * s + 128], lhsT=ks,
                                     rhs=qs, start=True, stop=True)
                nc.scalar.activation(e[:, 512 * h:512 * h + 512], dT[:],
                                     AF.Exp, scale=SCALE)
            # softmax sums broadcast to all partitions via ones-weights matmul
            # "bcyg" tag shared with the finish-loop yg: bc calls dominate
            # front phases, yg calls dominate finish phases, so bufs=2 gives
            # each true double-buffering in its own phase at no bank cost.
            bc = psum.tile([P, 512], F32, tag="bcyg", bufs=2, name="bc")
            for h in range(2):
                nc.tensor.matmul(bc[64 * h:64 * h + 64, :], lhsT=ones64[:],
                                 rhs=e[:, 512 * h:512 * h + 512],
                                 start=True, stop=True, tile_position=(0, 64 * h))
            nc.vector.tensor_copy(sums_st[:, g, :], bc[:])
            # oT[d, i] = sum_j v[j, d] e[j, i] ; head h in partition half h
            oT = psum.tile([P, 512], F32, tag="oT", bufs=1, name="oT")
            for s in range(4):
                for h in range(2):
                    nc.tensor.matmul(
                        oT[64 * h:64 * h + 64, 128 * s:128 * s + 128],
                        lhsT=vs[:, 128 * s + 64 * h:128 * s + 64 * h + 64],
                        rhs=e[:, 512 * h + 128 * s:512 * h + 128 * s + 128],
                        start=True, stop=True, tile_position=(0, 64 * h))
            nc.vector.tensor_copy(oTs_v[:, g], oT[:])

        def finish_group(g, wo, first_dir):
            # 1/sums = exp(-ln(sums)); ln already applied in the batched pass
            rcp = work.tile([P, 512], F32, tag="rcp", bufs=2)
            nc.scalar.activation(rcp[:], sums_st[:, g, :], AF.Exp, scale=-1.0)
            og = work.tile([P, 512], BF16, tag="og", bufs=2)
            nc.vector.tensor_mul(og[:], oTs_v[:, g], rcp[:])
            yg = psum.tile([P, 512], F32, tag="bcyg", bufs=2, name="yg")
            nc.tensor.matmul(yg[:], lhsT=wo[:], rhs=og[:], start=True, stop=True)
            if first_dir:
                # acc = yg + (bout_h + bout_w), contiguous write
                nc.vector.tensor_scalar_add(acc_f[:, 512 * g:512 * (g + 1)],
                                            yg[:], bsum[:, 0:1])
            else:
                # accumulate transposed: acc[:, i, 4g+s] += yg[:, (s, i)]
                yg_r = yg[:].rearrange("c (s i) -> c i s", s=4)
                acc_sl = acc[:, :, 4 * g:4 * g + 4]
                nc.vector.tensor_add(acc_sl, acc_sl, yg_r)

        sums_f = sums_st[:].rearrange("c g i -> c (g i)")

        def attn_direction(src_v, m01, wv, wo, first_dir):
            for g in range(32):
                front_group(g, src_v[:, 4 * g:4 * g + 4, :], m01, wv)
            # ONE ln instruction over all 32 groups' sums: a single ACT table
            # switch per direction (a per-group ln interleaves with the front
            # exps and thrashes the table set, ~1.5us per switch)
            nc.scalar.activation(sums_f, sums_f, AF.Ln)
            for g in range(32):  # exp(-ln): same table set as the front exps
                finish_group(g, wo, first_dir)

        # ---- phase 2: W-direction attention (rows of xp, contiguous) ----
        attn_direction(xp, m_w, wv_w, wo_w, True)

        # ---- xpT = xp with h/w swapped (DVE fast copy mode). Emitted after
        # the W-front so it does not steal DVE slots from the pool maxes that
        # feed the W ramp; it overlaps the W finish instead. ----
        for j in range(4):
            src = xp[:, :, 32 * j:32 * j + 32].rearrange("c h w -> c w h")
            nc.vector.tensor_copy(xpT_v[:, 32 * j:32 * j + 32, :], src)

        # ---- phase 3: H-direction attention (rows of xpT, contiguous) ----
        attn_direction(xpT_v, m_h, wv_h, wo_h, False)

        # ---- phase 3.5: relu over acc ----
        for j in range(4):
            sl = acc_f[:, 4096 * j:4096 * (j + 1)]
            nc.vector.tensor_scalar_max(sl, sl, 0.0)

        # ---- phase 4: conv pass, relu into ystore (bf16), stats ----
        # ystore layout: (c, chunk, eh, 512) so one relu covers both halves.
        y2_v = S[:].rearrange("c (n e i) -> c n e i", e=2, i=512)
        bnb = [stats.tile([P, 32, 6], F32, name=f"bnb{i}") for i in range(2)]
        for p in range(NPOS // 512):
            pos = slice(512 * p, 512 * (p + 1))
            for eh in range(2):
                yps = psum.tile([P, 512], F32, tag=("dT" if eh else "mkps"),
                                bufs=2, name="yps")
                ce = slice(128 * eh, 128 * eh + 128)
                nc.tensor.matmul(yps[:], lhsT=convA[:, ce], rhs=acc_f[:, pos],
                                 start=True, stop=False)
                nc.tensor.matmul(yps[:], lhsT=convX[:, ce], rhs=xp_f[:, pos],
                                 start=False, stop=True)
                nc.scalar.activation(y2_v[:, p, eh], yps[:], AF.Relu)
                nc.vector.bn_stats(bnb[eh][:, p, :], y2_v[:, p, eh])

        # ---- phase 5: aggregate stats, AllReduce, BN affine coefficients ----
        mv = stats.tile([P, 2, 2], F32)
        for eh in range(2):
            nc.vector.bn_aggr(mv[:, eh, :], bnb[eh][:])
        cc_in = stats.tile([P, 4], F32)
        for eh in range(2):
            # [mean, E[y^2]] per half; E[y^2] = var + mean^2
            nc.vector.tensor_copy(cc_in[:, 2 * eh:2 * eh + 1], mv[:, eh, 0:1])
            nc.vector.scalar_tensor_tensor(
                cc_in[:, 2 * eh + 1:2 * eh + 2],
                in0=mv[:, eh, 0:1], scalar=mv[:, eh, 0:1], in1=mv[:, eh, 1:2],
                op0=AluOpType.mult, op1=AluOpType.add)
        nc.sync.dma_start(out=stats_in_d, in_=cc_in[:])
        nc.gpsimd.collective_compute(
            "AllReduce", AluOpType.add,
            replica_groups=[list(range(NCORES))],
            ins=[stats_in_d], outs=[stats_out_d])
        gst = stats.tile([P, 4], F32)
        nc.sync.dma_start(out=gst[:], in_=stats_out_d)

        t0 = stats.tile([P, 4], F32)
        nc.vector.tensor_scalar_mul(t0[:], gst[:], 1.0 / NCORES)
        t0v = t0[:].rearrange("c (e two) -> c e two", two=2)
        m2 = stats.tile([P, 2], F32)
        veps = stats.tile([P, 2], F32)
        for eh in range(2):
            nc.vector.tensor_mul(m2[:, eh:eh + 1], t0v[:, eh, 0:1], t0v[:, eh, 0:1])
            nc.vector.scalar_tensor_tensor(
                veps[:, eh:eh + 1],
                in0=t0v[:, eh, 1:2], scalar=BN_EPS, in1=m2[:, eh:eh + 1],
                op0=AluOpType.add, op1=AluOpType.subtract)
        sd = stats.tile([P, 2], F32)
        nc.scalar.sqrt(sd[:], veps[:])
        rstd = stats.tile([P, 2], F32)
        nc.vector.reciprocal(rstd[:], sd[:])
        scl = stats.tile([P, 2], F32)
        nc.vector.tensor_mul(scl[:], gamma2[:], rstd[:])
        msc = stats.tile([P, 2], F32)
        means = stats.tile([P, 2], F32)
        nc.vector.tensor_copy(means[:, 0:1], t0v[:, 0, 0:1])
        nc.vector.tensor_copy(means[:, 1:2], t0v[:, 1, 0:1])
        nc.vector.tensor_mul(msc[:], means[:], scl[:])
        shift = stats.tile([P, 2], F32)
        nc.vector.tensor_sub(shift[:], beta2[:], msc[:])

        # ---- phase 6: affine from ystore, output (DVE/ACT/GPSIMD rotation) ----
        out_r = out_d.rearrange("(two c) h w -> two c (h w)", two=2)
        unit = 0
        for p in range(NPOS // 512):
            for eh in range(2):
                pos = slice(512 * p, 512 * (p + 1))
                ych = y2_v[:, p, eh]
                yo = work.tile([P, 512], F32, tag="yo", bufs=3)
                if unit % 2 == 0:
                    nc.vector.tensor_scalar(
                        yo[:], ych, scl[:, eh:eh + 1], shift[:, eh:eh + 1],
                        op0=AluOpType.mult, op1=AluOpType.add)
                else:
                    nc.scalar.activation(yo[:], ych, AF.Identity,
                                         bias=shift[:, eh:eh + 1],
                                         scale=scl[:, eh:eh + 1])
                nc.sync.dma_start(out=out_r[eh, :, pos], in_=yo[:])
                unit += 1

    nc.finalize()
    return nc


def _get_program():
    if "nc" not in _CACHE:
        _CACHE["nc"] = _build_program()
    return _CACHE["nc"]


def _make_in_maps(x, Wq_h, Wkv_h, Wout_h, bout_h, Wq_w, Wkv_w, Wout_w, bout_w,
                  conv_w, gamma, beta):
    import ml_dtypes
    f = np.float32
    bf = ml_dtypes.bfloat16

    shared = {
        "wq_w": np.ascontiguousarray(np.asarray(Wq_w, f).astype(bf)),
        "wk_w": np.ascontiguousarray(np.asarray(Wkv_w, f)[:, :C].astype(bf)),
        "wq_h": np.ascontiguousarray(np.asarray(Wq_h, f).astype(bf)),
        "wk_h": np.ascontiguousarray(np.asarray(Wkv_h, f)[:, :C].astype(bf)),
        "wv_w": np.ascontiguousarray(np.asarray(Wkv_w, f)[:, C:].astype(bf)),
        "wo_w": np.ascontiguousarray(np.asarray(Wout_w, f).astype(bf)),
        "wv_h": np.ascontiguousarray(np.asarray(Wkv_h, f)[:, C:].astype(bf)),
        "wo_h": np.ascontiguousarray(np.asarray(Wout_h, f).astype(bf)),
        "bsum": np.ascontiguousarray((np.asarray(bout_h, f) + np.asarray(bout_w, f)).reshape(C, 1)),
        "convA": np.ascontiguousarray(np.asarray(conv_w, f)[:C, :].astype(bf)),
        "convX": np.ascontiguousarray(np.asarray(conv_w, f)[C:, :].astype(bf)),
        "gamma2": np.ascontiguousarray(np.asarray(gamma, f).reshape(2, C).T),
        "beta2": np.ascontiguousarray(np.asarray(beta, f).reshape(2, C).T),
    }
    xb = np.asarray(x, f).astype(bf)
    return [{**shared, "x": np.ascontiguousarray(xb[b])} for b in range(B)]


def run(trace=False, **inputs):
    from concourse.bass_utils import run_bass_kernel_spmd

    nc = _get_program()
    in_maps = _make_in_maps(**inputs)
    res = run_bass_kernel_spmd(nc, in_maps, list(range(NCORES)), trace=trace)
    out = np.stack([res.results[b]["out"] for b in range(B)], axis=0)
    return out, res


def kernel(**inputs):
    out, _ = run(trace=False, **inputs)
    return out
